# revision 27
# baseline (speedup 1.0000x reference)
# Laplacian normalization kernel for Trainium2 (8 NeuronCores, SPMD).
#
# out = d^-1/2[:, None] * A * d^-1/2[None, :],  d_i = sum_j A[i, j],  A: [8192, 8192] f32
#
# Sharding: row-wise across 8 cores (1024 rows each). Row sums are local; the
# column-scale vector needs the full d^-1/2 [8192], via a tiny bf16 AllGather.
#
# The rel-err budget (2e-2) is ~25x looser than what bf16 costs (~4 roundings
# x 2^-9 ~ 0.8% worst case), so the whole kernel trades precision for HBM
# traffic, which is the binding constraint (measured ~330 GB/s/core vs the
# ~358 GB/s HBM-per-NC ceiling):
#   pass 1: stream the 32MB f32 shard once in 1MB chunks; one fused ACT op
#           per chunk casts it into a persistent bf16 SBUF cache (16MB: the
#           ENTIRE shard fits, f32 only fit half) and emits the row-sum via
#           accum_out. Stream-slot recycling is gated only on this single
#           ACT op (1.7us/chunk < 3us DMA pace), DVE stays off the load path.
#   middle: d^-1/2 = ACT sqrt + DVE reciprocal ([128,8]); PE-transpose to
#           [8,128] so the collective input is one contiguous 2KB bf16 DMA;
#           AllGather(2KB); row-scaling (DVE tensor_scalar 4x bf16 mode,
#           in-place on the cache) is placed after the collective in program
#           order but depends only on local dinv, so it fills the otherwise
#           dead ~30us collective window.
#   pass 2: broadcast the gathered bf16 vector across partitions (4 chunked
#           DMAs), then per 1MB chunk: DVE tensor_mul by cvec (2x bf16 mode,
#           in-place on the cache) and store bf16. No pass-2 HBM reads at
#           all; output stores are 16MB instead of 32MB.
#
# Per-core HBM traffic: 32MB read + 16MB write + ~2.5MB bcast/collective
# (vs 54.5MB read + 33.6MB write for the all-f32 two-pass version).
#
# SBUF/partition: 8x16KB bf16 cache + 5x8KB f32 stream + 16KB cvec + ~1KB
# small = ~185KB of the ~208KB Tile exposes.

import numpy as np

N = 8192
NCORES = 8
R = N // NCORES  # 1024 rows per core
P = 128          # SBUF partitions
T = R // P       # 8 row-tiles of [128, 8192] per core
NCHUNK = 4       # pass-1 column chunks per row-tile (1MB f32 each)
W = N // NCHUNK  # pass-1 chunk width (2048 columns)
SCHUNK = 2       # pass-2 column chunks per row-tile (1MB bf16 each)
SW = N // SCHUNK # pass-2 chunk width (4096 columns)
BCH = 4          # cvec broadcast chunks
BW = N // BCH

_cache = {}


def _build():
    import concourse.bacc as bacc
    import concourse.mybir as mybir
    import concourse.tile as tile
    from concourse import masks

    f32 = mybir.dt.float32
    bf16 = mybir.dt.bfloat16
    X = mybir.AxisListType.X
    mult = mybir.AluOpType.mult

    nc = bacc.Bacc(
        "TRN2", target_bir_lowering=False, debug=False, num_devices=NCORES
    )
    a = nc.dram_tensor("a_shard", [R, N], f32, kind="ExternalInput").ap()
    out = nc.dram_tensor("out_shard", [R, N], bf16, kind="ExternalOutput").ap()

    a_t = a.rearrange("(t p) n -> t p n", p=P)
    o_t = out.rearrange("(t p) n -> t p n", p=P)

    with tile.TileContext(nc) as tc:
        with (
            tc.tile_pool(name="cpool", bufs=1) as cpool,
            tc.tile_pool(name="spool", bufs=7) as spool,
            tc.tile_pool(name="vpool", bufs=1) as vpool,
            tc.tile_pool(name="psum", bufs=1, space="PSUM") as psum,
            tc.tile_pool(name="dram", bufs=1, space="DRAM") as dram,
        ):
            cache = {
                t: cpool.tile([P, N], bf16, tag=f"c{t}", name=f"c{t}")
                for t in range(T)
            }
            cvec = vpool.tile([P, N], bf16, tag="cvec")
            hpart = vpool.tile([P, NCHUNK * T + 1], f32, tag="hpart")
            dsum = vpool.tile([P, T], f32, tag="dsum")
            dinv = vpool.tile([P, T], f32, tag="dinv")
            ident = vpool.tile([P, P], f32, tag="ident")
            dinv_tp = vpool.tile([T, P], bf16, tag="dinv_tp")
            dinv_tpp = psum.tile([T, P], f32, tag="dinv_tpp")
            dloc = dram.tile([1, R], bf16, tag="dloc")
            dfull = dram.tile([1, N], bf16, tag="dfull")
            cwarm_i = dram.tile([1, 8], f32, tag="cwarm_i")
            cwarm_o = dram.tile([1, 8 * NCORES], f32, tag="cwarm_o")

            masks.make_identity(nc, ident[:, :])

            # warm up the gpsimd SWDGE path early: the first SWDGE dma_start
            # measured 18.7us (Q7 descriptor-ring init); pay it here, hidden
            # under pass-1 loads, so the dloc write before the collective is
            # cheap
            warm = dram.tile([1, P], f32, tag="warm", name="warm")
            nc.gpsimd.dma_start(out=warm[0, :], in_=ident[0:1, :])

            # ...and warm up the ncfw collective path with a dummy 16B
            # AllGather (the first CC op after the runtime barrier measured
            # ~13-16us of ncfw wakeup before data moved); it completes long
            # before the real one is issued
            nc.gpsimd.dma_start(out=cwarm_i[0, :], in_=ident[0:1, 0:8])
            nc.gpsimd.collective_compute(
                "AllGather",
                mybir.AluOpType.bypass,
                replica_groups=[list(range(NCORES))],
                ins=[cwarm_i[0, :].opt()],
                outs=[cwarm_o[0, :].opt()],
            )

            # pass 1: per 1MB chunk, ACT does a pure Copy cast into the bf16
            # cache (2.1us/chunk < the 2.8us DMA pace, so stream-slot
            # recycling never throttles the loads; the fused cast+accum
            # variant measured 2.7us/chunk and paced the whole pass), and the
            # otherwise-idle DVE row-sums the bf16 cache behind it. Loads
            # alternate between the two HWDGE queues.
            ld = [nc.sync, nc.scalar]
            nld = 0
            c = 0
            for t in range(T):
                if t < T - 1:
                    widths = [W] * NCHUNK
                else:
                    # taper the very last chunks so the post-load serial
                    # chain (last cast+reduce -> d^-1/2 -> collective) starts
                    # ~3us earlier
                    widths = [W, W, W, 3 * W // 4, W // 4]
                c0_tile = c
                col = 0
                for h, w in enumerate(widths):
                    cols = slice(col, col + w)
                    col += w
                    stile = spool.tile([P, W], f32, tag="s")
                    ld[nld % 2].dma_start(
                        out=stile[:, 0:w], in_=a_t[t][:, cols]
                    )
                    nld += 1
                    # cast f32 stream -> bf16 cache: every 4th chunk rides
                    # DVE (tensor_copy, 2x single-src mode, 1.1us) instead of
                    # ACT (2.1us) so ACT never becomes the pass pacer
                    if h % 4 == 3:
                        nc.vector.tensor_copy(cache[t][:, cols], stile[:, 0:w])
                    else:
                        nc.scalar.copy(cache[t][:, cols], stile[:, 0:w])
                    # row-sum reads the f32 STREAM, not the bf16 cache (bf16
                    # tensor_reduce has no packed uop, ~0.5 elem/cycle)
                    nc.vector.reduce_sum(
                        out=hpart[:, c : c + 1], in_=stile[:, 0:w], axis=X
                    )
                    c += 1
                nc.vector.reduce_sum(
                    out=dsum[:, t : t + 1],
                    in_=hpart[:, c0_tile:c],
                    axis=X,
                )

            # d^-1/2 (ACT Rsqrt is banned for accuracy; sqrt + DVE reciprocal),
            # then PE-transpose [128, T] -> [T, 128] so the collective input
            # DMA is one contiguous row-ordered 2KB write. The PSUM->SBUF
            # bf16 cast rides DVE (not ACT) to avoid an ACT table switch.
            nc.scalar.sqrt(dsum[:, :], dsum[:, :])
            nc.vector.reciprocal(dinv[:, :], dsum[:, :])
            nc.tensor.transpose(dinv_tpp[:, :], dinv[:, :], ident[:, :])
            nc.vector.tensor_copy(dinv_tp[:, :], dinv_tpp[:, :])
            # dloc written as 8 single-partition 256B DMAs: SWDGE desc-gen
            # costs ~1.9us PER partition-descriptor, so the one-shot [8,128]
            # write measured 15.5us on the critical pre-collective chain;
            # eight 1-descriptor writes are ~0.6us each (~4.5us serial on Q7)
            for t in range(T):
                nc.gpsimd.dma_start(
                    out=dloc[0, t * P : (t + 1) * P], in_=dinv_tp[t : t + 1, :]
                )

            nc.gpsimd.collective_compute(
                "AllGather",
                mybir.AluOpType.bypass,
                replica_groups=[list(range(NCORES))],
                ins=[dloc[0, :].opt()],
                outs=[dfull[0, :].opt()],
            )

            # row scaling: in-place on the bf16 cache, DVE 4x mode (measured
            # 2.8us/tile). Placed after the collective in program order but
            # gated only on local dinv -> fills the dead collective-wakeup
            # window on DVE. (Folding it into pass-2 STT was tried: the f32
            # scalar AP drops STT to 1x and the tail becomes DVE-paced.)
            for t in range(T):
                nc.vector.tensor_scalar_mul(
                    cache[t][:, :], cache[t][:, :], dinv[:, t : t + 1]
                )

            # replicate the gathered bf16 vector across all 128 partitions,
            # chunked and split across both HWDGE queues (everything after
            # this point is collective-gated anyway, so fan-out is all that
            # matters)
            bq = [nc.scalar, nc.sync]
            for h in range(BCH):
                cols = slice(h * BW, (h + 1) * BW)
                bq[h % 2].dma_start(
                    out=cvec[:, cols],
                    in_=dfull[0:1, cols].to_broadcast((P, BW)),
                )

            # pass 2: column scale via DVE tensor_mul (2x bf16 mode, measured
            # 2.75us/chunk), in-place on the cache, then store bf16.
            st = [nc.scalar, nc.sync]
            nst = 0
            for t in range(T):
                for h in range(SCHUNK):
                    cols = slice(h * SW, (h + 1) * SW)
                    nc.vector.tensor_mul(
                        cache[t][:, cols], cache[t][:, cols], cvec[:, cols]
                    )
                    st[nst % 2].dma_start(
                        out=o_t[t][:, cols], in_=cache[t][:, cols]
                    )
                    nst += 1

    nc.compile()
    return nc


def kernel(adjacency_matrix, _trace=False):
    from concourse.bass_utils import run_bass_kernel_spmd

    A = np.ascontiguousarray(np.asarray(adjacency_matrix, dtype=np.float32))
    assert A.shape == (N, N), A.shape

    if "nc" not in _cache:
        _cache["nc"] = _build()
    nc = _cache["nc"]

    in_maps = [{"a_shard": A[c * R : (c + 1) * R]} for c in range(NCORES)]
    res = run_bass_kernel_spmd(
        nc, in_maps, core_ids=list(range(NCORES)), trace=_trace
    )
    _cache["last"] = res
    return np.concatenate(
        [res.results[c]["out_shard"].astype(np.float32) for c in range(NCORES)],
        axis=0,
    )


# revision 30
# speedup vs baseline: 1.0384x; 1.0384x over previous
# Laplacian normalization kernel for Trainium2 (8 NeuronCores, SPMD).
#
# out = d^-1/2[:, None] * A * d^-1/2[None, :],  d_i = sum_j A[i, j],  A: [8192, 8192] f32
#
# Sharding: row-wise across 8 cores (1024 rows each). Row sums are local; the
# column-scale vector needs the full d^-1/2 [8192], via a tiny bf16 AllGather.
#
# The rel-err budget (2e-2) is ~25x looser than what bf16 costs (~4 roundings
# x 2^-9 ~ 0.8% worst case), so the whole kernel trades precision for HBM
# traffic, which is the binding constraint (measured ~330 GB/s/core vs the
# ~358 GB/s HBM-per-NC ceiling):
#   pass 1: stream the 32MB f32 shard once in 1MB chunks; one fused ACT op
#           per chunk casts it into a persistent bf16 SBUF cache (16MB: the
#           ENTIRE shard fits, f32 only fit half) and emits the row-sum via
#           accum_out. Stream-slot recycling is gated only on this single
#           ACT op (1.7us/chunk < 3us DMA pace), DVE stays off the load path.
#   middle: d^-1/2 = ACT sqrt + DVE reciprocal ([128,8]); PE-transpose to
#           [8,128] so the collective input is one contiguous 2KB bf16 DMA;
#           AllGather(2KB); row-scaling (DVE tensor_scalar 4x bf16 mode,
#           in-place on the cache) is placed after the collective in program
#           order but depends only on local dinv, so it fills the otherwise
#           dead ~30us collective window.
#   pass 2: broadcast the gathered bf16 vector across partitions (4 chunked
#           DMAs), then per 1MB chunk: DVE tensor_mul by cvec (2x bf16 mode,
#           in-place on the cache) and store bf16. No pass-2 HBM reads at
#           all; output stores are 16MB instead of 32MB.
#
# Per-core HBM traffic: 32MB read + 16MB write + ~2.5MB bcast/collective
# (vs 54.5MB read + 33.6MB write for the all-f32 two-pass version).
#
# SBUF/partition: 8x16KB bf16 cache + 5x8KB f32 stream + 16KB cvec + ~1KB
# small = ~185KB of the ~208KB Tile exposes.

import numpy as np

N = 8192
NCORES = 8
R = N // NCORES  # 1024 rows per core
P = 128          # SBUF partitions
T = R // P       # 8 row-tiles of [128, 8192] per core
NCHUNK = 4       # pass-1 column chunks per row-tile (1MB f32 each)
W = N // NCHUNK  # pass-1 chunk width (2048 columns)
SCHUNK = 2       # pass-2 column chunks per row-tile (1MB bf16 each)
SW = N // SCHUNK # pass-2 chunk width (4096 columns)
BCH = 4          # cvec broadcast chunks
BW = N // BCH

_cache = {}


def _build():
    import concourse.bacc as bacc
    import concourse.mybir as mybir
    import concourse.tile as tile
    from concourse import masks

    f32 = mybir.dt.float32
    bf16 = mybir.dt.bfloat16
    X = mybir.AxisListType.X
    mult = mybir.AluOpType.mult

    nc = bacc.Bacc(
        "TRN2", target_bir_lowering=False, debug=False, num_devices=NCORES
    )
    a = nc.dram_tensor("a_shard", [R, N], f32, kind="ExternalInput").ap()
    out = nc.dram_tensor("out_shard", [R, N], bf16, kind="ExternalOutput").ap()

    a_t = a.rearrange("(t p) n -> t p n", p=P)
    o_t = out.rearrange("(t p) n -> t p n", p=P)

    with tile.TileContext(nc) as tc:
        with (
            tc.tile_pool(name="cpool", bufs=1) as cpool,
            tc.tile_pool(name="spool", bufs=7) as spool,
            tc.tile_pool(name="vpool", bufs=1) as vpool,
            tc.tile_pool(name="psum", bufs=1, space="PSUM") as psum,
            tc.tile_pool(name="dram", bufs=1, space="DRAM") as dram,
        ):
            cache = {
                t: cpool.tile([P, N], bf16, tag=f"c{t}", name=f"c{t}")
                for t in range(T)
            }
            cvec = vpool.tile([P, N], bf16, tag="cvec")
            hpart = vpool.tile([P, NCHUNK * T + 1], f32, tag="hpart")
            dsum = vpool.tile([P, T], f32, tag="dsum")
            dinv = vpool.tile([P, T], f32, tag="dinv")
            ident = vpool.tile([P, P], f32, tag="ident")
            dinv_tp = vpool.tile([T, P], bf16, tag="dinv_tp")
            dinv_tpp = psum.tile([T, P], f32, tag="dinv_tpp")
            dloc = dram.tile([1, R], bf16, tag="dloc")
            dfull = dram.tile([1, N], bf16, tag="dfull")
            cwarm_i = dram.tile([1, 8], f32, tag="cwarm_i")
            cwarm_o = dram.tile([1, 8 * NCORES], f32, tag="cwarm_o")

            masks.make_identity(nc, ident[:, :])

            # warm up the gpsimd SWDGE path early: the first SWDGE dma_start
            # measured 18.7us (Q7 descriptor-ring init); pay it here, hidden
            # under pass-1 loads, so the dloc write before the collective is
            # cheap
            warm = dram.tile([1, P], f32, tag="warm", name="warm")
            nc.gpsimd.dma_start(out=warm[0, :], in_=ident[0:1, :])

            # ...and warm up the ncfw collective path with a dummy 16B
            # AllGather (the first CC op after the runtime barrier measured
            # ~13-16us of ncfw wakeup before data moved); it completes long
            # before the real one is issued
            nc.gpsimd.dma_start(out=cwarm_i[0, :], in_=ident[0:1, 0:8])
            nc.gpsimd.collective_compute(
                "AllGather",
                mybir.AluOpType.bypass,
                replica_groups=[list(range(NCORES))],
                ins=[cwarm_i[0, :].opt()],
                outs=[cwarm_o[0, :].opt()],
            )

            # pass 1: per 1MB chunk, ACT does a pure Copy cast into the bf16
            # cache (2.1us/chunk < the 2.8us DMA pace, so stream-slot
            # recycling never throttles the loads; the fused cast+accum
            # variant measured 2.7us/chunk and paced the whole pass), and the
            # otherwise-idle DVE row-sums the bf16 cache behind it. Loads
            # alternate between the two HWDGE queues.
            ld = [nc.sync, nc.scalar]
            nld = 0
            c = 0
            for t in range(T):
                if t < T - 1:
                    widths = [W] * NCHUNK
                else:
                    # taper the very last chunks so the post-load serial
                    # chain (last cast+reduce -> d^-1/2 -> collective) starts
                    # ~3us earlier
                    widths = [W, W, W, 3 * W // 4, W // 4]
                c0_tile = c
                col = 0
                for h, w in enumerate(widths):
                    cols = slice(col, col + w)
                    col += w
                    stile = spool.tile([P, W], f32, tag="s")
                    ld[nld % 2].dma_start(
                        out=stile[:, 0:w], in_=a_t[t][:, cols]
                    )
                    nld += 1
                    # cast f32 stream -> bf16 cache: every 4th chunk rides
                    # DVE (tensor_copy, 2x single-src mode, 1.1us) instead of
                    # ACT (2.1us) so ACT never becomes the pass pacer
                    if h % 4 == 3:
                        nc.vector.tensor_copy(cache[t][:, cols], stile[:, 0:w])
                    else:
                        nc.scalar.copy(cache[t][:, cols], stile[:, 0:w])
                    # row-sum reads the f32 STREAM, not the bf16 cache (bf16
                    # tensor_reduce has no packed uop, ~0.5 elem/cycle)
                    nc.vector.reduce_sum(
                        out=hpart[:, c : c + 1], in_=stile[:, 0:w], axis=X
                    )
                    c += 1
                nc.vector.reduce_sum(
                    out=dsum[:, t : t + 1],
                    in_=hpart[:, c0_tile:c],
                    axis=X,
                )

            # d^-1/2 (ACT Rsqrt is banned for accuracy; sqrt + DVE reciprocal),
            # then PE-transpose [128, T] -> [T, 128] so the collective input
            # DMA is one contiguous row-ordered 2KB write. The PSUM->SBUF
            # bf16 cast rides DVE (not ACT) to avoid an ACT table switch.
            nc.scalar.sqrt(dsum[:, :], dsum[:, :])
            nc.vector.reciprocal(dinv[:, :], dsum[:, :])
            nc.tensor.transpose(dinv_tpp[:, :], dinv[:, :], ident[:, :])
            nc.vector.tensor_copy(dinv_tp[:, :], dinv_tpp[:, :])
            # dloc written as 8 single-partition 256B DMAs: SWDGE desc-gen
            # costs ~1.9us PER partition-descriptor, so the one-shot [8,128]
            # write measured 15.5us on the critical pre-collective chain;
            # eight 1-descriptor writes are ~0.6us each (~4.5us serial on Q7)
            for t in range(T):
                nc.gpsimd.dma_start(
                    out=dloc[0, t * P : (t + 1) * P], in_=dinv_tp[t : t + 1, :]
                )

            nc.gpsimd.collective_compute(
                "AllGather",
                mybir.AluOpType.bypass,
                replica_groups=[list(range(NCORES))],
                ins=[dloc[0, :].opt()],
                outs=[dfull[0, :].opt()],
            )

            # row scaling: in-place on the bf16 cache, gated only on local
            # dinv -> fills the collective window. Split 4/4 between DVE
            # (tensor_scalar 4x, 2.8us/tile) and ACT (Copy-activation with
            # per-partition scale, 6.8us/tile): with all 8 on DVE the queue
            # reached the first pass-2 tensor_mul ~10us AFTER the AllGather
            # landed on the last-arriving core. (Folding the scale into
            # pass-2 STT was tried: STT has no fast-mode uops and the tail
            # went DVE-bound.)
            dve_tiles = [0, 2, 4, 6]
            act_tiles = [1, 3, 5, 7]
            for t in dve_tiles:
                nc.vector.tensor_scalar_mul(
                    cache[t][:, :], cache[t][:, :], dinv[:, t : t + 1]
                )
            for t in act_tiles:
                nc.scalar.mul(
                    cache[t][:, :], cache[t][:, :], dinv[:, t : t + 1]
                )

            # replicate the gathered bf16 vector across all 128 partitions,
            # chunked and split across both HWDGE queues (everything after
            # this point is collective-gated anyway, so fan-out is all that
            # matters)
            # (tail DMAs all ride the sync queue: the scalar queue is busy
            # with the ACT row-scales, and HWDGE issue is in-order per queue)
            for h in range(BCH):
                cols = slice(h * BW, (h + 1) * BW)
                nc.sync.dma_start(
                    out=cvec[:, cols],
                    in_=dfull[0:1, cols].to_broadcast((P, BW)),
                )

            # pass 2: column scale via DVE tensor_mul (2x bf16 mode, measured
            # 2.75us/chunk), in-place on the cache, then store bf16. Tiles
            # ordered DVE-scaled first so the TT stream never waits on the
            # slower ACT row-scales.
            for t in dve_tiles + act_tiles:
                for h in range(SCHUNK):
                    cols = slice(h * SW, (h + 1) * SW)
                    nc.vector.tensor_mul(
                        cache[t][:, cols], cache[t][:, cols], cvec[:, cols]
                    )
                    nc.sync.dma_start(
                        out=o_t[t][:, cols], in_=cache[t][:, cols]
                    )

    nc.compile()
    return nc


def kernel(adjacency_matrix, _trace=False):
    from concourse.bass_utils import run_bass_kernel_spmd

    A = np.ascontiguousarray(np.asarray(adjacency_matrix, dtype=np.float32))
    assert A.shape == (N, N), A.shape

    if "nc" not in _cache:
        _cache["nc"] = _build()
    nc = _cache["nc"]

    in_maps = [{"a_shard": A[c * R : (c + 1) * R]} for c in range(NCORES)]
    res = run_bass_kernel_spmd(
        nc, in_maps, core_ids=list(range(NCORES)), trace=_trace
    )
    _cache["last"] = res
    return np.concatenate(
        [res.results[c]["out_shard"].astype(np.float32) for c in range(NCORES)],
        axis=0,
    )
